# revision 15
# baseline (speedup 1.0000x reference)
"""Causal self-attention Trainium2 Bass kernel.

Problem: B=4, T=2048, DIM=1024, H=16 heads, head_dim=64 (fp32).
  qkv = x @ w_qkv.T ; per-head causal softmax(q k^T / 8) v ; out @ w_out.T

Sharding (8 cores): core c -> (batch b = c//2, head-group g = c%2 of 8 heads).
Each core computes a partial output y_partial = attn_out_g @ w_out[:, g]^T
for its batch; host sums the two head-group partials per batch.

Device layout (per core):
  xt      [1024, 2048] bf16 : x[b]^T (dim-major)          -- host-transposed
  wqkvt   [1024, 1536] bf16 : [Wq|Wk|Wv]^T slice          -- host-transposed
  woutt   [ 512, 1024] bf16 : w_out[:, g]^T               -- host-transposed
  tri     [ 128,  128] bf16 : keep-triangle (1.0 iff q >= k within a block)
  yt      [1024, 2048] bf16 : partial output, transposed

Pipeline per token-chunk c (512 tokens), fully interleaved so PE keeps busy
while ScalarE runs the exp stream:
  1. QKV projection -> QT/KT (head-dim major, bf16) and V (token major, bf16,
     with a ones column per head that makes P@V also emit the softmax
     denominator row).
  2. Attention for q-chunk c: transposed scores for 2 heads per PSUM quad
     (row-packed via base_partition 0/64 so the K=64 matmuls run
     concurrently); one exp on ScalarE (scale=1/8 folded in, no
     max-subtraction; |scores| small so fp32 exp is safe); the 4 diagonal
     ktiles use partial-width scores/exp/PV (only q >= 128j columns) plus a
     128x128 triangle mask multiply, so ~53% instead of 62.5% of the full
     T^2 work is done; P@V accumulates per-head output plus denominator row;
     divide via fast-reciprocal off PSUM + GpSimd partition-broadcast +
     vector multiply (no DMA round-trip).
  3. Output projection of the finished 512-token chunk (bf16 out).
"""

import contextlib
import functools
import itertools

import numpy as np
import ml_dtypes

import concourse.bass as bass
import concourse.mybir as mybir
import concourse.tile as tile
from concourse import bacc
from concourse.bass_utils import run_bass_kernel_spmd

B, T, DIM = 4, 2048, 1024
NUM_HEADS, HEAD_DIM = 16, 64
INNER = NUM_HEADS * HEAD_DIM
SCALE = HEAD_DIM ** -0.5

N_CORES = 8
HEADS_PER_CORE = 8
HG = HEADS_PER_CORE * HEAD_DIM  # 512 = inner slice per core
NCH = T // 512                  # 4 token chunks
KT_PER_CH = 4                   # 128-ktok tiles per 512 chunk

F32 = mybir.dt.float32
BF16 = mybir.dt.bfloat16


def build_bass():
    nc = bacc.Bacc()
    xt = nc.declare_dram_parameter("xt", [DIM, T], BF16, isOutput=False)
    wqkvt = nc.declare_dram_parameter("wqkvt", [DIM, 3 * HG], BF16, isOutput=False)
    woutt = nc.declare_dram_parameter("woutt", [HG, DIM], BF16, isOutput=False)
    tri = nc.declare_dram_parameter("tri", [128, 128], BF16, isOutput=False)
    yt = nc.declare_dram_parameter("yt", [DIM, T], BF16, isOutput=True)

    with tile.TileContext(nc) as tc:
        _emit(nc, tc, xt, wqkvt, woutt, tri, yt)
    nc.finalize()
    return nc


def _emit(nc, tc, xt, wqkvt, woutt, tri, yt):
    ctx = contextlib.ExitStack()
    with ctx:
        singles = ctx.enter_context(tc.tile_pool(name="singles", bufs=1))
        xpool = ctx.enter_context(tc.tile_pool(name="xpool", bufs=16))
        epool = ctx.enter_context(tc.tile_pool(name="epool", bufs=4))
        apool = ctx.enter_context(tc.tile_pool(name="apool", bufs=2))
        spool = ctx.enter_context(tc.tile_pool(name="spool", bufs=1))
        # PSUM budget (8 banks of 2KB/partition):
        #   pair [128,1024] bufs=2 -> 4 banks (scores, double-buffered)
        #   ot   [65,512]  2 slots -> 2 banks (otA/otB of the live pair; the
        #     next pair's PV start waits on this pair's divide, which the
        #     filler matmuls between pairs hide)
        #   qkv  [128,512] bufs=2  -> 2 banks (stage 1/3 groups double-buffered
        #     so group N+1's matmuls overlap group N's PSUM->SBUF copy)
        psq = ctx.enter_context(tc.tile_pool(name="psq", bufs=2, space="PSUM"))
        psot = ctx.enter_context(tc.tile_pool(name="psot", bufs=2, space="PSUM"))
        psmm = ctx.enter_context(tc.tile_pool(name="psmm", bufs=2, space="PSUM"))

        # ---- persistent SBUF tensors (wq first: they gate the first matmul).
        # wq goes down the Sync HWDGE queue, x chunk-0 down the Scalar HWDGE
        # queue, interleaved per k-slice so the k-step accumulation can start
        # as soon as the first slices land.
        wq = [singles.tile([128, 3 * HG], BF16, name=f"wq{k}") for k in range(8)]

        # QT/KT: 4 tiles [128, 2048] (2 heads per tile, head-dim major)
        qt = [singles.tile([128, T], BF16, name=f"qt{m}") for m in range(4)]
        kt = [singles.tile([128, T], BF16, name=f"kt{m}") for m in range(4)]
        # V: 16 token-tiles [128, 8*65] bf16 (per head: 64 v-cols + ones col)
        vt = [singles.tile([128, HEADS_PER_CORE * 65], BF16, name=f"vt{t}")
              for t in range(16)]

        def stage1_x(c, eng=None):
            cs = slice(c * 512, (c + 1) * 512)
            xts = []
            for k in range(8):
                xtile = xpool.tile([128, 512], BF16, tag="xt", name=f"x{c}_{k}")
                (eng or nc.sync).dma_start(out=xtile, in_=xt[k * 128:(k + 1) * 128, cs])
                xts.append(xtile)
            return xts

        def stage1_part(c, xts, m):
            cs = slice(c * 512, (c + 1) * 512)
            for which, dst in ((0, qt), (1, kt)):
                ps = psmm.tile([128, 512], F32, tag="qkv", name=f"pq{c}{which}{m}")
                for k in range(8):
                    nc.tensor.matmul(
                        ps,
                        lhsT=wq[k][:, which * HG + m * 128: which * HG + (m + 1) * 128],
                        rhs=xts[k],
                        start=(k == 0), stop=(k == 7),
                    )
                nc.vector.tensor_copy(dst[m][:, cs], ps)
            t = c * 4 + m
            ps = psmm.tile([128, 512], F32, tag="qkv", name=f"pv{t}")
            for k in range(8):
                nc.tensor.matmul(
                    ps,
                    lhsT=xts[k][:, m * 128:(m + 1) * 128],
                    rhs=wq[k][:, 2 * HG:3 * HG],
                    start=(k == 0), stop=(k == 7),
                )
            v3 = vt[t].rearrange("p (h d) -> p h d", h=HEADS_PER_CORE)
            nc.vector.tensor_copy(
                v3[:, :, 0:64],
                ps.rearrange("p (h d) -> p h d", h=HEADS_PER_CORE))

        # startup: alternate wq and x chunk 0 over both HWDGE queues
        # (Sync + Scalar) so the k-step gating transfers land fastest
        xts0 = []
        for k in range(8):
            weng = nc.sync if k % 2 == 0 else nc.scalar
            xeng = nc.scalar if k % 2 == 0 else nc.sync
            weng.dma_start(out=wq[k], in_=wqkvt[k * 128:(k + 1) * 128, :])
            xtile = xpool.tile([128, 512], BF16, tag="xt", name=f"x0_{k}")
            xeng.dma_start(out=xtile, in_=xt[k * 128:(k + 1) * 128, 0:512])
            xts0.append(xtile)
        for m in range(4):
            stage1_part(0, xts0, m)

        trib = singles.tile([128, 128], BF16, name="trib")
        nc.scalar.dma_start(out=trib, in_=tri[:, :])
        wo = []
        for k in range(4):
            w = singles.tile([128, DIM], BF16, name=f"wo{k}")
            nc.scalar.dma_start(out=w, in_=woutt[k * 128:(k + 1) * 128, :])
            wo.append(w)
        # ones columns of V (denominator trick), written on-chip
        for t in range(16):
            v3 = vt[t].rearrange("p (h d) -> p h d", h=HEADS_PER_CORE)
            nc.vector.memset(v3[:, :, 64:65], 1.0)

        def attention(c, fillers, aot):
            """Emit attention for chunk c; after each head-pair, emit the next
            filler chunk (stage1/stage3 matmul groups) so the PE has queued
            work while ScalarE drains the exp backlog for that pair."""
            n_kt = KT_PER_CH * (c + 1)
            fillers = list(fillers)
            per_pair = (len(fillers) + 3) // 4 if fillers else 0
            for hp in range(4):            # head pair (2hp, 2hp+1)
                hA, hB = 2 * hp, 2 * hp + 1
                otA = psot.tile([65, 512], F32, tag="ot", name=f"otA{c}_{hp}")
                otB = psot.tile([65, 512], F32, tag="ot", name=f"otB{c}_{hp}")
                for tk in range(n_kt):  # pair: ktile tk x 2 heads
                    q = psq.tile([128, 1024], F32, tag="pair", name=f"s{c}_{hp}_{tk}")
                    # last 4 ktiles hit the causal diagonal: only columns
                    # q >= 128j are live -> partial-width scores/exp/PV
                    diag = tk >= n_kt - 4
                    j = tk - (n_kt - 4) if diag else 0
                    q0 = 128 * j            # first live q col within the chunk
                    nq = 512 - q0
                    for i in range(2):
                        ho = i * 64
                        nc.tensor.matmul(
                            q[:, i * 512 + q0:(i + 1) * 512],
                            lhsT=kt[hp][ho:ho + 64, tk * 128:(tk + 1) * 128],
                            rhs=qt[hp][ho:ho + 64, c * 512 + q0:(c + 1) * 512],
                            start=True, stop=True,
                            tile_position=(ho, 0),
                        )
                    e = epool.tile([128, 1024], BF16, tag="e", name=f"e{c}_{hp}_{tk}")
                    e3 = e.rearrange("p (h q) -> p h q", h=2)
                    q3 = q.rearrange("p (h q) -> p h q", h=2)
                    nc.scalar.activation(e3[:, :, q0:512], q3[:, :, q0:512],
                                         mybir.ActivationFunctionType.Exp,
                                         scale=float(SCALE))
                    if diag:
                        for i in range(2):
                            blk = slice(i * 512 + q0, i * 512 + q0 + 128)
                            nc.vector.tensor_mul(e[:, blk], e[:, blk], trib)
                    for i, h in ((0, hA), (1, hB)):
                        nc.tensor.matmul(
                            (otA if i == 0 else otB)[:, q0:512],
                            lhsT=vt[tk][:, h * 65:h * 65 + 65],
                            rhs=e[:, i * 512 + q0:(i + 1) * 512],
                            start=(tk == 0), stop=(tk == n_kt - 1),
                        )
                for (h, ot) in ((hA, otA), (hB, otB)):
                    den = spool.tile([1, 512], F32, tag="den", name=f"dn{c}_{h}")
                    if c == NCH - 1 and hp == 3:
                        # ScalarE has drained its exp queue by now; shorten
                        # the tail-critical divide chain by copying there
                        nc.scalar.copy(den, ot[64:65, :])
                    else:
                        nc.vector.tensor_copy(den, ot[64:65, :])
                    recf = spool.tile([1, 512], F32, tag="recf", name=f"rf{c}_{h}")
                    nc.vector.reciprocal_approx_fast(recf, den)
                    rb = spool.tile([64, 512], F32, tag="rb", bufs=2,
                                    name=f"rb{c}_{h}")
                    nc.gpsimd.partition_broadcast(rb, recf, channels=64)
                    nc.vector.tensor_mul(
                        aot[hp][(h % 2) * 64:(h % 2) * 64 + 64, :],
                        ot[0:64, :], rb)
                for _ in range(per_pair):
                    if fillers:
                        fillers.pop(0)()
            while fillers:
                fillers.pop(0)()
            return aot

        def stage3_part(c, aot, i):
            cs = slice(c * 512, (c + 1) * 512)
            for od in (2 * i, 2 * i + 1):
                ps = psmm.tile([128, 512], F32, tag="qkv", name=f"py{c}_{od}")
                for k in range(4):
                    nc.tensor.matmul(
                        ps,
                        lhsT=wo[k][:, od * 128:(od + 1) * 128],
                        rhs=aot[k],
                        start=(k == 0), stop=(k == 3),
                    )
                ys = spool.tile([128, 512], BF16, tag="ys", bufs=2, name=f"ys{c}_{od}")
                nc.vector.tensor_copy(ys, ps)
                # final chunk: ScalarE is done with exps; split the output
                # stream over both HWDGE queues to drain the tail faster
                eng = nc.scalar if (c == NCH - 1 and od % 2 == 1) else nc.sync
                eng.dma_start(out=yt[od * 128:(od + 1) * 128, cs], in_=ys)

        # Final-chunk output projection, split by contraction halves: the
        # k={0,1} half only needs head-pairs 0/1 of attention(NCH-1), so it
        # runs as filler behind pairs 2/3; only the k={2,3} half plus a DVE
        # combine remains after the last pair's divide.
        yp = [spool.tile([128, 512], F32, tag=f"yp{od}", name=f"yp{od}")
              for od in range(8)]

        def stage3_h1(c, aot, i):
            for od in (2 * i, 2 * i + 1):
                ps = psmm.tile([128, 512], F32, tag="qkv", name=f"pyA{c}_{od}")
                for k in (0, 1):
                    nc.tensor.matmul(
                        ps,
                        lhsT=wo[k][:, od * 128:(od + 1) * 128],
                        rhs=aot[k],
                        start=(k == 0), stop=(k == 1),
                    )
                nc.vector.tensor_copy(yp[od], ps)

        def stage3_h2(c, aot, i):
            cs = slice(c * 512, (c + 1) * 512)
            for od in (2 * i, 2 * i + 1):
                ps = psmm.tile([128, 512], F32, tag="qkv", name=f"pyB{c}_{od}")
                for k in (2, 3):
                    nc.tensor.matmul(
                        ps,
                        lhsT=wo[k][:, od * 128:(od + 1) * 128],
                        rhs=aot[k],
                        start=(k == 2), stop=(k == 3),
                    )
                ys = spool.tile([128, 512], BF16, tag="ys", bufs=2, name=f"ys{c}_{od}")
                nc.vector.scalar_tensor_tensor(
                    ys, ps, 1.0, yp[od],
                    op0=mybir.AluOpType.mult, op1=mybir.AluOpType.add)
                eng = nc.scalar if od % 2 == 1 else nc.sync
                eng.dma_start(out=yt[od * 128:(od + 1) * 128, cs], in_=ys)

        # stage1(c+1) / stage3(c-1) matmul groups are interleaved between the
        # head-pairs of attention(c): the PE then always has dense projection
        # work queued while ScalarE drains each pair's exp stream (which is
        # ~1.7x slower than the PE's score/PV production rate).
        aot_prev = None
        for c in range(NCH):
            aot = [apool.tile([128, 512], BF16, tag=f"aot{k}", name=f"aot{c}_{k}")
                   for k in range(4)]
            f1, f3 = [], []
            if c + 1 < NCH:
                xts = stage1_x(c + 1)
                f1 = [functools.partial(stage1_part, c + 1, xts, m)
                      for m in range(4)]
            if aot_prev is not None:
                f3 = [functools.partial(stage3_part, c - 1, aot_prev, i)
                      for i in range(4)]
            if c == NCH - 1:
                # s3(2) parts behind pairs 0/1, the k01-half of s3(3) behind
                # pairs 2/3 (it only reads aot[0], aot[1])
                fillers = f3 + [functools.partial(stage3_h1, c, aot, i)
                                for i in range(4)]
            else:
                fillers = [f for pair in itertools.zip_longest(f1, f3)
                           for f in pair if f is not None]
            attention(c, fillers, aot)
            aot_prev = aot
        for i in range(4):
            stage3_h2(NCH - 1, aot_prev, i)


_NC_CACHE = None


def _get_nc():
    global _NC_CACHE
    if _NC_CACHE is None:
        _NC_CACHE = build_bass()
    return _NC_CACHE


def make_tri():
    """Keep-triangle for the 128-wide diagonal blocks: tri[k, q] = 1 iff q >= k."""
    k = np.arange(128)[:, None]
    q = np.arange(128)[None, :]
    return (q >= k).astype(ml_dtypes.bfloat16)


def make_in_maps(x, w_qkv, w_out):
    x = np.asarray(x, dtype=np.float32)
    w_qkv = np.asarray(w_qkv, dtype=np.float32)
    w_out = np.asarray(w_out, dtype=np.float32)
    tri = make_tri()
    in_maps = []
    for c in range(N_CORES):
        b, g = c // 2, c % 2
        gs = slice(g * HG, (g + 1) * HG)
        wsel = np.concatenate(
            [w_qkv[0 * INNER:][gs], w_qkv[1 * INNER:][gs], w_qkv[2 * INNER:][gs]],
            axis=0)                               # [1536, 1024]
        in_maps.append({
            "xt": np.ascontiguousarray(x[b].T).astype(ml_dtypes.bfloat16),
            "wqkvt": np.ascontiguousarray(wsel.T).astype(ml_dtypes.bfloat16),
            "woutt": np.ascontiguousarray(w_out[:, gs].T).astype(ml_dtypes.bfloat16),
            "tri": tri,
        })
    return in_maps


def kernel(x, mask, w_qkv, w_out, **_):
    nc = _get_nc()
    in_maps = make_in_maps(x, w_qkv, w_out)
    res = run_bass_kernel_spmd(nc, in_maps, core_ids=list(range(N_CORES)))
    y = np.zeros((B, T, DIM), dtype=np.float32)
    for c in range(N_CORES):
        y[c // 2] += res.results[c]["yt"].astype(np.float32).T
    return y


# revision 23
# speedup vs baseline: 1.0409x; 1.0409x over previous
"""Causal self-attention Trainium2 Bass kernel.

Problem: B=4, T=2048, DIM=1024, H=16 heads, head_dim=64 (fp32).
  qkv = x @ w_qkv.T ; per-head causal softmax(q k^T / 8) v ; out @ w_out.T

Sharding (8 cores): core c -> (batch b = c//2, head-group g = c%2 of 8 heads).
Each core computes a partial output y_partial = attn_out_g @ w_out[:, g]^T
for its batch; host sums the two head-group partials per batch.

Device layout (per core):
  xt      [1024, 2048] bf16 : x[b]^T (dim-major)          -- host-transposed
  wqkvt   [1024, 1536] bf16 : [Wq|Wk|Wv]^T slice          -- host-transposed
  woutt   [ 512, 1024] bf16 : w_out[:, g]^T               -- host-transposed
  tri     [ 128,  128] bf16 : keep-triangle (1.0 iff q >= k within a block)
  yt      [1024, 2048] bf16 : partial output, transposed

Pipeline per token-chunk c (512 tokens), fully interleaved so PE keeps busy
while ScalarE runs the exp stream:
  1. QKV projection -> QT/KT (head-dim major, bf16) and V (token major, bf16,
     with a ones column per head that makes P@V also emit the softmax
     denominator row).
  2. Attention for q-chunk c: transposed scores for 2 heads per PSUM quad
     (row-packed via base_partition 0/64 so the K=64 matmuls run
     concurrently); one exp on ScalarE (scale=1/8 folded in, no
     max-subtraction; |scores| small so fp32 exp is safe); the 4 diagonal
     ktiles use partial-width scores/exp/PV (only q >= 128j columns) plus a
     128x128 triangle mask multiply, so ~53% instead of 62.5% of the full
     T^2 work is done; P@V accumulates per-head output plus denominator row;
     divide via fast-reciprocal off PSUM + GpSimd partition-broadcast +
     vector multiply (no DMA round-trip).
  3. Output projection of the finished 512-token chunk (bf16 out).
"""

import contextlib
import functools
import itertools

import numpy as np
import ml_dtypes

import concourse.bass as bass
import concourse.mybir as mybir
import concourse.tile as tile
from concourse import bacc
from concourse.bass_utils import run_bass_kernel_spmd

B, T, DIM = 4, 2048, 1024
NUM_HEADS, HEAD_DIM = 16, 64
INNER = NUM_HEADS * HEAD_DIM
SCALE = HEAD_DIM ** -0.5

N_CORES = 8
HEADS_PER_CORE = 8
HG = HEADS_PER_CORE * HEAD_DIM  # 512 = inner slice per core
NCH = T // 512                  # 4 token chunks
KT_PER_CH = 4                   # 128-ktok tiles per 512 chunk

F32 = mybir.dt.float32
BF16 = mybir.dt.bfloat16


def build_bass():
    nc = bacc.Bacc()
    # x/y are chunk-major [NCH, DIM, 512] so every [128, 512] tile transfer
    # is one contiguous 128KB block (vs 128 strided 1KB rows)
    xt = nc.declare_dram_parameter("xt", [NCH, DIM, 512], BF16, isOutput=False)
    wqkvt = nc.declare_dram_parameter("wqkvt", [DIM, 3 * HG], BF16, isOutput=False)
    woutt = nc.declare_dram_parameter("woutt", [HG, DIM], BF16, isOutput=False)
    tri = nc.declare_dram_parameter("tri", [128, 128], BF16, isOutput=False)
    yt = nc.declare_dram_parameter("yt", [NCH, DIM, 512], BF16, isOutput=True)

    with tile.TileContext(nc) as tc:
        _emit(nc, tc, xt, wqkvt, woutt, tri, yt)
    nc.finalize()
    return nc


def _emit(nc, tc, xt, wqkvt, woutt, tri, yt):
    ctx = contextlib.ExitStack()
    with ctx:
        singles = ctx.enter_context(tc.tile_pool(name="singles", bufs=1))
        xpool = ctx.enter_context(tc.tile_pool(name="xpool", bufs=16))
        epool = ctx.enter_context(tc.tile_pool(name="epool", bufs=4))
        apool = ctx.enter_context(tc.tile_pool(name="apool", bufs=2))
        spool = ctx.enter_context(tc.tile_pool(name="spool", bufs=1))
        # PSUM budget (8 banks of 2KB/partition):
        #   pair [128,1024] bufs=2 -> 4 banks (scores, double-buffered)
        #   ot   [65,512]  2 slots -> 2 banks (otA/otB of the live pair; the
        #     next pair's PV start waits on this pair's divide, which the
        #     filler matmuls between pairs hide)
        #   qkv  [128,512] bufs=2  -> 2 banks (stage 1/3 groups double-buffered
        #     so group N+1's matmuls overlap group N's PSUM->SBUF copy)
        psq = ctx.enter_context(tc.tile_pool(name="psq", bufs=2, space="PSUM"))
        psot = ctx.enter_context(tc.tile_pool(name="psot", bufs=2, space="PSUM"))
        psmm = ctx.enter_context(tc.tile_pool(name="psmm", bufs=2, space="PSUM"))

        # ---- persistent SBUF tensors (wq first: they gate the first matmul).
        # wq goes down the Sync HWDGE queue, x chunk-0 down the Scalar HWDGE
        # queue, interleaved per k-slice so the k-step accumulation can start
        # as soon as the first slices land.
        wq = [singles.tile([128, 3 * HG], BF16, name=f"wq{k}") for k in range(8)]

        # QT/KT: 4 tiles [128, 2048] (2 heads per tile, head-dim major)
        qt = [singles.tile([128, T], BF16, name=f"qt{m}") for m in range(4)]
        kt = [singles.tile([128, T], BF16, name=f"kt{m}") for m in range(4)]
        # V: 16 token-tiles [128, 8*65] bf16 (per head: 64 v-cols + ones col)
        vt = [singles.tile([128, HEADS_PER_CORE * 65], BF16, name=f"vt{t}")
              for t in range(16)]

        def stage1_x(c, eng=None):
            xts = []
            for k in range(8):
                xtile = xpool.tile([128, 512], BF16, tag="xt", name=f"x{c}_{k}")
                (eng or nc.sync).dma_start(
                    out=xtile, in_=xt[c, k * 128:(k + 1) * 128, :])
                xts.append(xtile)
            return xts

        def stage1_part(c, xts, m):
            cs = slice(c * 512, (c + 1) * 512)
            for which, dst in ((0, qt), (1, kt)):
                ps = psmm.tile([128, 512], F32, tag="qkv", name=f"pq{c}{which}{m}")
                for k in range(8):
                    nc.tensor.matmul(
                        ps,
                        lhsT=wq[k][:, which * HG + m * 128: which * HG + (m + 1) * 128],
                        rhs=xts[k],
                        start=(k == 0), stop=(k == 7),
                    )
                nc.vector.tensor_copy(dst[m][:, cs], ps)
            t = c * 4 + m
            ps = psmm.tile([128, 512], F32, tag="qkv", name=f"pv{t}")
            for k in range(8):
                nc.tensor.matmul(
                    ps,
                    lhsT=xts[k][:, m * 128:(m + 1) * 128],
                    rhs=wq[k][:, 2 * HG:3 * HG],
                    start=(k == 0), stop=(k == 7),
                )
            v3 = vt[t].rearrange("p (h d) -> p h d", h=HEADS_PER_CORE)
            nc.vector.tensor_copy(
                v3[:, :, 0:64],
                ps.rearrange("p (h d) -> p h d", h=HEADS_PER_CORE))

        # startup: alternate wq and x chunk 0 over both HWDGE queues
        # (Sync + Scalar) so the k-step gating transfers land fastest
        xts0 = []
        for k in range(8):
            weng = nc.sync if k % 2 == 0 else nc.scalar
            xeng = nc.scalar if k % 2 == 0 else nc.sync
            weng.dma_start(out=wq[k], in_=wqkvt[k * 128:(k + 1) * 128, :])
            xtile = xpool.tile([128, 512], BF16, tag="xt", name=f"x0_{k}")
            xeng.dma_start(out=xtile, in_=xt[0, k * 128:(k + 1) * 128, :])
            xts0.append(xtile)
        for m in range(4):
            stage1_part(0, xts0, m)

        trib = singles.tile([128, 128], BF16, name="trib")
        nc.scalar.dma_start(out=trib, in_=tri[:, :])
        wo = []
        for k in range(4):
            w = singles.tile([128, DIM], BF16, name=f"wo{k}")
            nc.scalar.dma_start(out=w, in_=woutt[k * 128:(k + 1) * 128, :])
            wo.append(w)
        # ones columns of V (denominator trick), written on-chip
        for t in range(16):
            v3 = vt[t].rearrange("p (h d) -> p h d", h=HEADS_PER_CORE)
            nc.vector.memset(v3[:, :, 64:65], 1.0)

        def attention(c, fillers, aot):
            """Emit attention for chunk c; after each head-pair, emit the next
            filler chunk (stage1/stage3 matmul groups) so the PE has queued
            work while ScalarE drains the exp backlog for that pair."""
            n_kt = KT_PER_CH * (c + 1)
            fillers = list(fillers)
            per_pair = (len(fillers) + 3) // 4 if fillers else 0
            for hp in range(4):            # head pair (2hp, 2hp+1)
                hA, hB = 2 * hp, 2 * hp + 1
                otA = psot.tile([65, 512], F32, tag="ot", name=f"otA{c}_{hp}")
                otB = psot.tile([65, 512], F32, tag="ot", name=f"otB{c}_{hp}")
                for tk in range(n_kt):  # pair: ktile tk x 2 heads
                    q = psq.tile([128, 1024], F32, tag="pair", name=f"s{c}_{hp}_{tk}")
                    # last 4 ktiles hit the causal diagonal: only columns
                    # q >= 128j are live -> partial-width scores/exp/PV
                    diag = tk >= n_kt - 4
                    j = tk - (n_kt - 4) if diag else 0
                    q0 = 128 * j            # first live q col within the chunk
                    nq = 512 - q0
                    for i in range(2):
                        ho = i * 64
                        nc.tensor.matmul(
                            q[:, i * 512 + q0:(i + 1) * 512],
                            lhsT=kt[hp][ho:ho + 64, tk * 128:(tk + 1) * 128],
                            rhs=qt[hp][ho:ho + 64, c * 512 + q0:(c + 1) * 512],
                            start=True, stop=True,
                            tile_position=(ho, 0),
                        )
                    e = epool.tile([128, 1024], BF16, tag="e", name=f"e{c}_{hp}_{tk}")
                    e3 = e.rearrange("p (h q) -> p h q", h=2)
                    q3 = q.rearrange("p (h q) -> p h q", h=2)
                    nc.scalar.activation(e3[:, :, q0:512], q3[:, :, q0:512],
                                         mybir.ActivationFunctionType.Exp,
                                         scale=float(SCALE))
                    if diag:
                        for i in range(2):
                            blk = slice(i * 512 + q0, i * 512 + q0 + 128)
                            nc.vector.tensor_mul(e[:, blk], e[:, blk], trib)
                    for i, h in ((0, hA), (1, hB)):
                        nc.tensor.matmul(
                            (otA if i == 0 else otB)[:, q0:512],
                            lhsT=vt[tk][:, h * 65:h * 65 + 65],
                            rhs=e[:, i * 512 + q0:(i + 1) * 512],
                            start=(tk == 0), stop=(tk == n_kt - 1),
                        )
                # reciprocal + broadcast now; the aot multiplies AFTER the
                # filler so the DVE never head-of-line blocks on the ~1us
                # GpSimd broadcast (the filler's matmuls run meanwhile)
                rbs = []
                for (h, ot) in ((hA, otA), (hB, otB)):
                    den = spool.tile([1, 512], F32, tag="den", name=f"dn{c}_{h}")
                    if c == NCH - 1 and hp == 3:
                        # ScalarE has drained its exp queue by now; shorten
                        # the tail-critical divide chain by copying there
                        nc.scalar.copy(den, ot[64:65, :])
                    else:
                        nc.vector.tensor_copy(den, ot[64:65, :])
                    recf = spool.tile([1, 512], F32, tag="recf", name=f"rf{c}_{h}")
                    nc.vector.reciprocal_approx_fast(recf, den)
                    rb = spool.tile([64, 512], F32, tag="rb", bufs=2,
                                    name=f"rb{c}_{h}")
                    nc.gpsimd.partition_broadcast(rb, recf, channels=64)
                    rbs.append((h, ot, rb))
                for _ in range(per_pair):
                    if fillers:
                        fillers.pop(0)()
                for (h, ot, rb) in rbs:
                    nc.vector.tensor_mul(
                        aot[hp][(h % 2) * 64:(h % 2) * 64 + 64, :],
                        ot[0:64, :], rb)
            while fillers:
                fillers.pop(0)()
            return aot

        def stage3_part(c, aot, i):
            for od in (2 * i, 2 * i + 1):
                ps = psmm.tile([128, 512], F32, tag="qkv", name=f"py{c}_{od}")
                for k in range(4):
                    nc.tensor.matmul(
                        ps,
                        lhsT=wo[k][:, od * 128:(od + 1) * 128],
                        rhs=aot[k],
                        start=(k == 0), stop=(k == 3),
                    )
                ys = spool.tile([128, 512], BF16, tag="ys", bufs=2, name=f"ys{c}_{od}")
                nc.vector.tensor_copy(ys, ps)
                nc.sync.dma_start(out=yt[c, od * 128:(od + 1) * 128, :], in_=ys)

        # Final-chunk output projection, split by contraction halves: the
        # k={0,1} half only needs head-pairs 0/1 of attention(NCH-1), so it
        # runs as filler behind pairs 2/3; only the k={2,3} half plus a DVE
        # combine remains after the last pair's divide.
        yp = [spool.tile([128, 512], F32, tag=f"yp{od}", name=f"yp{od}")
              for od in range(8)]

        def stage3_h1(c, aot, i):
            for od in (2 * i, 2 * i + 1):
                ps = psmm.tile([128, 512], F32, tag="qkv", name=f"pyA{c}_{od}")
                for k in (0, 1):
                    nc.tensor.matmul(
                        ps,
                        lhsT=wo[k][:, od * 128:(od + 1) * 128],
                        rhs=aot[k],
                        start=(k == 0), stop=(k == 1),
                    )
                nc.vector.tensor_copy(yp[od], ps)

        def stage3_h2(c, aot, i):
            cs = slice(c * 512, (c + 1) * 512)
            for od in (2 * i, 2 * i + 1):
                ps = psmm.tile([128, 512], F32, tag="qkv", name=f"pyB{c}_{od}")
                for k in (2, 3):
                    nc.tensor.matmul(
                        ps,
                        lhsT=wo[k][:, od * 128:(od + 1) * 128],
                        rhs=aot[k],
                        start=(k == 2), stop=(k == 3),
                    )
                ys = spool.tile([128, 512], BF16, tag="ys", bufs=2, name=f"ys{c}_{od}")
                nc.vector.scalar_tensor_tensor(
                    ys, ps, 1.0, yp[od],
                    op0=mybir.AluOpType.mult, op1=mybir.AluOpType.add)
                eng = nc.scalar if od % 2 == 1 else nc.sync
                eng.dma_start(out=yt[c, od * 128:(od + 1) * 128, :], in_=ys)

        # stage1(c+1) / stage3(c-1) matmul groups are interleaved between the
        # head-pairs of attention(c): the PE then always has dense projection
        # work queued while ScalarE drains each pair's exp stream (which is
        # ~1.7x slower than the PE's score/PV production rate).
        aot_prev = None
        for c in range(NCH):
            aot = [apool.tile([128, 512], BF16, tag=f"aot{k}", name=f"aot{c}_{k}")
                   for k in range(4)]
            f1, f3 = [], []
            if c + 1 < NCH:
                xts = stage1_x(c + 1)
                f1 = [functools.partial(stage1_part, c + 1, xts, m)
                      for m in range(4)]
            if aot_prev is not None:
                f3 = [functools.partial(stage3_part, c - 1, aot_prev, i)
                      for i in range(4)]
            if c == NCH - 1:
                # s3(2) parts behind pairs 0/1, the k01-half of s3(3) behind
                # pairs 2/3 (it only reads aot[0], aot[1])
                fillers = f3 + [functools.partial(stage3_h1, c, aot, i)
                                for i in range(4)]
            else:
                fillers = [f for pair in itertools.zip_longest(f1, f3)
                           for f in pair if f is not None]
            attention(c, fillers, aot)
            aot_prev = aot
        for i in range(4):
            stage3_h2(NCH - 1, aot_prev, i)


_NC_CACHE = None


def _get_nc():
    global _NC_CACHE
    if _NC_CACHE is None:
        _NC_CACHE = build_bass()
    return _NC_CACHE


def make_tri():
    """Keep-triangle for the 128-wide diagonal blocks: tri[k, q] = 1 iff q >= k."""
    k = np.arange(128)[:, None]
    q = np.arange(128)[None, :]
    return (q >= k).astype(ml_dtypes.bfloat16)


def make_in_maps(x, w_qkv, w_out):
    x = np.asarray(x, dtype=np.float32)
    w_qkv = np.asarray(w_qkv, dtype=np.float32)
    w_out = np.asarray(w_out, dtype=np.float32)
    tri = make_tri()
    in_maps = []
    for c in range(N_CORES):
        b, g = c // 2, c % 2
        gs = slice(g * HG, (g + 1) * HG)
        wsel = np.concatenate(
            [w_qkv[0 * INNER:][gs], w_qkv[1 * INNER:][gs], w_qkv[2 * INNER:][gs]],
            axis=0)                               # [1536, 1024]
        xtc = x[b].T.reshape(DIM, NCH, 512).transpose(1, 0, 2)  # [NCH, DIM, 512]
        in_maps.append({
            "xt": np.ascontiguousarray(xtc).astype(ml_dtypes.bfloat16),
            "wqkvt": np.ascontiguousarray(wsel.T).astype(ml_dtypes.bfloat16),
            "woutt": np.ascontiguousarray(w_out[:, gs].T).astype(ml_dtypes.bfloat16),
            "tri": tri,
        })
    return in_maps


def kernel(x, mask, w_qkv, w_out, **_):
    nc = _get_nc()
    in_maps = make_in_maps(x, w_qkv, w_out)
    res = run_bass_kernel_spmd(nc, in_maps, core_ids=list(range(N_CORES)))
    y = np.zeros((B, T, DIM), dtype=np.float32)
    for c in range(N_CORES):
        ytc = res.results[c]["yt"].astype(np.float32)  # [NCH, DIM, 512]
        y[c // 2] += ytc.transpose(0, 2, 1).reshape(T, DIM)
    return y


# revision 27
# speedup vs baseline: 1.0495x; 1.0082x over previous
"""Causal self-attention Trainium2 Bass kernel.

Problem: B=4, T=2048, DIM=1024, H=16 heads, head_dim=64 (fp32).
  qkv = x @ w_qkv.T ; per-head causal softmax(q k^T / 8) v ; out @ w_out.T

Sharding (8 cores): core c -> (batch b = c//2, head-group g = c%2 of 8 heads).
Each core computes a partial output y_partial = attn_out_g @ w_out[:, g]^T
for its batch; host sums the two head-group partials per batch.

Device layout (per core):
  xt      [1024, 2048] bf16 : x[b]^T (dim-major)          -- host-transposed
  wqkvt   [1024, 1536] bf16 : [Wq|Wk|Wv]^T slice          -- host-transposed
  woutt   [ 512, 1024] bf16 : w_out[:, g]^T               -- host-transposed
  tri     [ 128,  128] bf16 : keep-triangle (1.0 iff q >= k within a block)
  yt      [1024, 2048] bf16 : partial output, transposed

Pipeline per token-chunk c (512 tokens), fully interleaved so PE keeps busy
while ScalarE runs the exp stream:
  1. QKV projection -> QT/KT (head-dim major, bf16) and V (token major, bf16,
     with a ones column per head that makes P@V also emit the softmax
     denominator row).
  2. Attention for q-chunk c: transposed scores for 2 heads per PSUM quad
     (row-packed via base_partition 0/64 so the K=64 matmuls run
     concurrently); one exp on ScalarE (scale=1/8 folded in, no
     max-subtraction; |scores| small so fp32 exp is safe); the 4 diagonal
     ktiles use partial-width scores/exp/PV (only q >= 128j columns) plus a
     128x128 triangle mask multiply, so ~53% instead of 62.5% of the full
     T^2 work is done; P@V accumulates per-head output plus denominator row;
     divide via fast-reciprocal off PSUM + GpSimd partition-broadcast +
     vector multiply (no DMA round-trip).
  3. Output projection of the finished 512-token chunk (bf16 out).
"""

import contextlib
import functools
import itertools

import numpy as np
import ml_dtypes

import concourse.bass as bass
import concourse.mybir as mybir
import concourse.tile as tile
from concourse import bacc
from concourse.bass_utils import run_bass_kernel_spmd

B, T, DIM = 4, 2048, 1024
NUM_HEADS, HEAD_DIM = 16, 64
INNER = NUM_HEADS * HEAD_DIM
SCALE = HEAD_DIM ** -0.5

N_CORES = 8
HEADS_PER_CORE = 8
HG = HEADS_PER_CORE * HEAD_DIM  # 512 = inner slice per core
NCH = T // 512                  # 4 token chunks
KT_PER_CH = 4                   # 128-ktok tiles per 512 chunk

F32 = mybir.dt.float32
BF16 = mybir.dt.bfloat16


def build_bass():
    nc = bacc.Bacc()
    # x/y are chunk-major [NCH, DIM, 512] so every [128, 512] tile transfer
    # is one contiguous 128KB block (vs 128 strided 1KB rows)
    xt = nc.declare_dram_parameter("xt", [NCH, DIM, 512], BF16, isOutput=False)
    wqkvt = nc.declare_dram_parameter("wqkvt", [DIM, 3 * HG], BF16, isOutput=False)
    woutt = nc.declare_dram_parameter("woutt", [HG, DIM], BF16, isOutput=False)
    tri = nc.declare_dram_parameter("tri", [128, 128], BF16, isOutput=False)
    yt = nc.declare_dram_parameter("yt", [NCH, DIM, 512], BF16, isOutput=True)

    with tile.TileContext(nc) as tc:
        _emit(nc, tc, xt, wqkvt, woutt, tri, yt)
    nc.finalize()
    return nc


def _emit(nc, tc, xt, wqkvt, woutt, tri, yt):
    ctx = contextlib.ExitStack()
    with ctx:
        singles = ctx.enter_context(tc.tile_pool(name="singles", bufs=1))
        xpool = ctx.enter_context(tc.tile_pool(name="xpool", bufs=16))
        epool = ctx.enter_context(tc.tile_pool(name="epool", bufs=4))
        apool = ctx.enter_context(tc.tile_pool(name="apool", bufs=2))
        spool = ctx.enter_context(tc.tile_pool(name="spool", bufs=1))
        # PSUM budget (8 banks of 2KB/partition):
        #   pair [128,1024] bufs=2 -> 4 banks (scores, double-buffered)
        #   ot   [65,512]  2 slots -> 2 banks (otA/otB of the live pair; the
        #     next pair's PV start waits on this pair's divide, which the
        #     filler matmuls between pairs hide)
        #   qkv  [128,512] bufs=2  -> 2 banks (stage 1/3 groups double-buffered
        #     so group N+1's matmuls overlap group N's PSUM->SBUF copy)
        psq = ctx.enter_context(tc.tile_pool(name="psq", bufs=2, space="PSUM"))
        psot = ctx.enter_context(tc.tile_pool(name="psot", bufs=2, space="PSUM"))
        psmm = ctx.enter_context(tc.tile_pool(name="psmm", bufs=2, space="PSUM"))

        # ---- persistent SBUF tensors (wq first: they gate the first matmul).
        # wq goes down the Sync HWDGE queue, x chunk-0 down the Scalar HWDGE
        # queue, interleaved per k-slice so the k-step accumulation can start
        # as soon as the first slices land.
        wq = [singles.tile([128, 3 * HG], BF16, name=f"wq{k}") for k in range(8)]

        # QT/KT: 4 tiles [128, 2048] (2 heads per tile, head-dim major)
        qt = [singles.tile([128, T], BF16, name=f"qt{m}") for m in range(4)]
        kt = [singles.tile([128, T], BF16, name=f"kt{m}") for m in range(4)]
        # V: 16 token-tiles [128, 8*65] bf16 (per head: 64 v-cols + ones col)
        vt = [singles.tile([128, HEADS_PER_CORE * 65], BF16, name=f"vt{t}")
              for t in range(16)]

        def stage1_x(c, eng=None):
            xts = []
            for k in range(8):
                xtile = xpool.tile([128, 512], BF16, tag="xt", name=f"x{c}_{k}")
                (eng or nc.sync).dma_start(
                    out=xtile, in_=xt[c, k * 128:(k + 1) * 128, :])
                xts.append(xtile)
            return xts

        def stage1_qk(c, xts, which, m):
            cs = slice(c * 512, (c + 1) * 512)
            dst = qt if which == 0 else kt
            ps = psmm.tile([128, 512], F32, tag="qkv", name=f"pq{c}{which}{m}")
            for k in range(8):
                nc.tensor.matmul(
                    ps,
                    lhsT=wq[k][:, which * HG + m * 128: which * HG + (m + 1) * 128],
                    rhs=xts[k],
                    start=(k == 0), stop=(k == 7),
                )
            nc.vector.tensor_copy(dst[m][:, cs], ps)

        def stage1_v(c, xts, m):
            t = c * 4 + m
            ps = psmm.tile([128, 512], F32, tag="qkv", name=f"pv{t}")
            for k in range(8):
                nc.tensor.matmul(
                    ps,
                    lhsT=xts[k][:, m * 128:(m + 1) * 128],
                    rhs=wq[k][:, 2 * HG:3 * HG],
                    start=(k == 0), stop=(k == 7),
                )
            v3 = vt[t].rearrange("p (h d) -> p h d", h=HEADS_PER_CORE)
            nc.vector.tensor_copy(
                v3[:, :, 0:64],
                ps.rearrange("p (h d) -> p h d", h=HEADS_PER_CORE))

        def stage1_part(c, xts, m):
            stage1_qk(c, xts, 0, m)
            stage1_qk(c, xts, 1, m)
            stage1_v(c, xts, m)

        # startup: alternate wq and x chunk 0 over both HWDGE queues
        # (Sync + Scalar) so the k-step gating transfers land fastest
        xts0 = []
        for k in range(8):
            weng = nc.sync if k % 2 == 0 else nc.scalar
            xeng = nc.scalar if k % 2 == 0 else nc.sync
            weng.dma_start(out=wq[k], in_=wqkvt[k * 128:(k + 1) * 128, :])
            xtile = xpool.tile([128, 512], BF16, tag="xt", name=f"x0_{k}")
            xeng.dma_start(out=xtile, in_=xt[0, k * 128:(k + 1) * 128, :])
            xts0.append(xtile)
        for m in range(4):
            stage1_part(0, xts0, m)

        trib = singles.tile([128, 128], BF16, name="trib")
        nc.scalar.dma_start(out=trib, in_=tri[:, :])
        wo = []
        for k in range(4):
            w = singles.tile([128, DIM], BF16, name=f"wo{k}")
            nc.scalar.dma_start(out=w, in_=woutt[k * 128:(k + 1) * 128, :])
            wo.append(w)
        # ones columns of V (denominator trick), written on-chip
        for t in range(16):
            v3 = vt[t].rearrange("p (h d) -> p h d", h=HEADS_PER_CORE)
            nc.vector.memset(v3[:, :, 64:65], 1.0)

        def attention(c, units, aot, cadence):
            """Emit attention for chunk c. `units` is a list of (min_pair, fn)
            small PE work units (single stage1/stage3 PSUM groups); one is
            injected every `cadence` ktiles so the PE always has dense matmul
            work queued while ScalarE's exp stream (983ns/tile) lags the
            score/PV production rate (~640ns/tile). PV is emitted one ktile
            behind scores so the PE never in-order-stalls on exp(tk)."""
            n_kt = KT_PER_CH * (c + 1)
            units = list(units)

            def pop_unit(hp):
                for idx, (mp, fn) in enumerate(units):
                    if mp <= hp:
                        units.pop(idx)
                        fn()
                        return True
                return False

            for hp in range(4):            # head pair (2hp, 2hp+1)
                hA, hB = 2 * hp, 2 * hp + 1
                otA = psot.tile([65, 512], F32, tag="ot", name=f"otA{c}_{hp}")
                otB = psot.tile([65, 512], F32, tag="ot", name=f"otB{c}_{hp}")
                pend = None       # (tk, e, q0) -> PV deferred one ktile
                since = 0

                def emit_pv(pv):
                    tk, e, q0 = pv
                    for i, ot in ((0, otA), (1, otB)):
                        nc.tensor.matmul(
                            ot[:, q0:512],
                            lhsT=vt[tk][:, (2 * hp + i) * 65:(2 * hp + i) * 65 + 65],
                            rhs=e[:, i * 512 + q0:(i + 1) * 512],
                            start=(tk == 0), stop=(tk == n_kt - 1),
                        )

                for tk in range(n_kt):  # pair: ktile tk x 2 heads
                    q = psq.tile([128, 1024], F32, tag="pair", name=f"s{c}_{hp}_{tk}")
                    # last 4 ktiles hit the causal diagonal: only columns
                    # q >= 128j are live -> partial-width scores/exp/PV
                    diag = tk >= n_kt - 4
                    j = tk - (n_kt - 4) if diag else 0
                    q0 = 128 * j            # first live q col within the chunk
                    for i in range(2):
                        ho = i * 64
                        nc.tensor.matmul(
                            q[:, i * 512 + q0:(i + 1) * 512],
                            lhsT=kt[hp][ho:ho + 64, tk * 128:(tk + 1) * 128],
                            rhs=qt[hp][ho:ho + 64, c * 512 + q0:(c + 1) * 512],
                            start=True, stop=True,
                            tile_position=(ho, 0),
                        )
                    e = epool.tile([128, 1024], BF16, tag="e", name=f"e{c}_{hp}_{tk}")
                    e3 = e.rearrange("p (h q) -> p h q", h=2)
                    q3 = q.rearrange("p (h q) -> p h q", h=2)
                    nc.scalar.activation(e3[:, :, q0:512], q3[:, :, q0:512],
                                         mybir.ActivationFunctionType.Exp,
                                         scale=float(SCALE))
                    if diag:
                        for i in range(2):
                            blk = slice(i * 512 + q0, i * 512 + q0 + 128)
                            nc.vector.tensor_mul(e[:, blk], e[:, blk], trib)
                    if pend is not None:
                        emit_pv(pend)
                    pend = (tk, e, q0)
                    since += 1
                    if since >= cadence and tk < n_kt - 1:
                        if pop_unit(hp):
                            since = 0
                emit_pv(pend)
                # reciprocal + broadcast now; the aot multiplies AFTER a
                # filler unit so the DVE never head-of-line blocks on the ~1us
                # GpSimd broadcast (the filler's matmuls run meanwhile)
                rbs = []
                for (h, ot) in ((hA, otA), (hB, otB)):
                    den = spool.tile([1, 512], F32, tag="den", name=f"dn{c}_{h}")
                    if c == NCH - 1 and hp == 3:
                        # ScalarE has drained its exp queue by now; shorten
                        # the tail-critical divide chain by copying there
                        nc.scalar.copy(den, ot[64:65, :])
                    else:
                        nc.vector.tensor_copy(den, ot[64:65, :])
                    recf = spool.tile([1, 512], F32, tag="recf", name=f"rf{c}_{h}")
                    nc.vector.reciprocal_approx_fast(recf, den)
                    rb = spool.tile([64, 512], F32, tag="rb", bufs=2,
                                    name=f"rb{c}_{h}")
                    nc.gpsimd.partition_broadcast(rb, recf, channels=64)
                    rbs.append((h, ot, rb))
                pop_unit(hp)
                for (h, ot, rb) in rbs:
                    nc.vector.tensor_mul(
                        aot[hp][(h % 2) * 64:(h % 2) * 64 + 64, :],
                        ot[0:64, :], rb)
            while units:
                units.pop(0)[1]()
            return aot

        def stage3_od(c, aot, od):
            ps = psmm.tile([128, 512], F32, tag="qkv", name=f"py{c}_{od}")
            for k in range(4):
                nc.tensor.matmul(
                    ps,
                    lhsT=wo[k][:, od * 128:(od + 1) * 128],
                    rhs=aot[k],
                    start=(k == 0), stop=(k == 3),
                )
            ys = spool.tile([128, 512], BF16, tag="ys", bufs=2, name=f"ys{c}_{od}")
            nc.vector.tensor_copy(ys, ps)
            nc.sync.dma_start(out=yt[c, od * 128:(od + 1) * 128, :], in_=ys)

        # Final-chunk output projection, split by contraction halves: the
        # k={0,1} half only needs head-pairs 0/1 of attention(NCH-1), so it
        # runs as filler behind pairs 2/3; only the k={2,3} half plus a DVE
        # combine remains after the last pair's divide.
        yp = [spool.tile([128, 512], F32, tag=f"yp{od}", name=f"yp{od}")
              for od in range(8)]

        def stage3_h1(c, aot, i):
            for od in (2 * i, 2 * i + 1):
                ps = psmm.tile([128, 512], F32, tag="qkv", name=f"pyA{c}_{od}")
                for k in (0, 1):
                    nc.tensor.matmul(
                        ps,
                        lhsT=wo[k][:, od * 128:(od + 1) * 128],
                        rhs=aot[k],
                        start=(k == 0), stop=(k == 1),
                    )
                nc.vector.tensor_copy(yp[od], ps)

        def stage3_h2(c, aot, i):
            cs = slice(c * 512, (c + 1) * 512)
            for od in (2 * i, 2 * i + 1):
                ps = psmm.tile([128, 512], F32, tag="qkv", name=f"pyB{c}_{od}")
                for k in (2, 3):
                    nc.tensor.matmul(
                        ps,
                        lhsT=wo[k][:, od * 128:(od + 1) * 128],
                        rhs=aot[k],
                        start=(k == 2), stop=(k == 3),
                    )
                ys = spool.tile([128, 512], BF16, tag="ys", bufs=2, name=f"ys{c}_{od}")
                nc.vector.scalar_tensor_tensor(
                    ys, ps, 1.0, yp[od],
                    op0=mybir.AluOpType.mult, op1=mybir.AluOpType.add)
                eng = nc.scalar if od % 2 == 1 else nc.sync
                eng.dma_start(out=yt[c, od * 128:(od + 1) * 128, :], in_=ys)

        # stage1(c+1) / stage3(c-1) PSUM groups are injected between ktiles of
        # attention(c) (cadence tuned per chunk so the filler rate matches the
        # exp-vs-PE deficit). h1 units carry min_pair=2: they read aot[0]/[1]
        # of the CURRENT chunk, which exist only once pairs 0/1 are divided.
        aot_prev = None
        cadence = [2, 2, 3, 4]
        for c in range(NCH):
            aot = [apool.tile([128, 512], BF16, tag=f"aot{k}", name=f"aot{c}_{k}")
                   for k in range(4)]
            u1, u3 = [], []
            if c + 1 < NCH:
                xts = stage1_x(c + 1)
                for m in range(4):
                    u1.append((0, functools.partial(stage1_qk, c + 1, xts, 0, m)))
                    u1.append((0, functools.partial(stage1_qk, c + 1, xts, 1, m)))
                    u1.append((0, functools.partial(stage1_v, c + 1, xts, m)))
            if aot_prev is not None:
                u3 = [(0, functools.partial(stage3_od, c - 1, aot_prev, od))
                      for od in range(8)]
            if c == NCH - 1:
                u3 += [(2, functools.partial(stage3_h1, c, aot, i))
                       for i in range(4)]
            units = [u for pair in itertools.zip_longest(u1, u3)
                     for u in pair if u is not None]
            attention(c, units, aot, cadence[c])
            aot_prev = aot
        for i in range(4):
            stage3_h2(NCH - 1, aot_prev, i)


_NC_CACHE = None


def _get_nc():
    global _NC_CACHE
    if _NC_CACHE is None:
        _NC_CACHE = build_bass()
    return _NC_CACHE


def make_tri():
    """Keep-triangle for the 128-wide diagonal blocks: tri[k, q] = 1 iff q >= k."""
    k = np.arange(128)[:, None]
    q = np.arange(128)[None, :]
    return (q >= k).astype(ml_dtypes.bfloat16)


def make_in_maps(x, w_qkv, w_out):
    x = np.asarray(x, dtype=np.float32)
    w_qkv = np.asarray(w_qkv, dtype=np.float32)
    w_out = np.asarray(w_out, dtype=np.float32)
    tri = make_tri()
    in_maps = []
    for c in range(N_CORES):
        b, g = c // 2, c % 2
        gs = slice(g * HG, (g + 1) * HG)
        wsel = np.concatenate(
            [w_qkv[0 * INNER:][gs], w_qkv[1 * INNER:][gs], w_qkv[2 * INNER:][gs]],
            axis=0)                               # [1536, 1024]
        xtc = x[b].T.reshape(DIM, NCH, 512).transpose(1, 0, 2)  # [NCH, DIM, 512]
        in_maps.append({
            "xt": np.ascontiguousarray(xtc).astype(ml_dtypes.bfloat16),
            "wqkvt": np.ascontiguousarray(wsel.T).astype(ml_dtypes.bfloat16),
            "woutt": np.ascontiguousarray(w_out[:, gs].T).astype(ml_dtypes.bfloat16),
            "tri": tri,
        })
    return in_maps


def kernel(x, mask, w_qkv, w_out, **_):
    nc = _get_nc()
    in_maps = make_in_maps(x, w_qkv, w_out)
    res = run_bass_kernel_spmd(nc, in_maps, core_ids=list(range(N_CORES)))
    y = np.zeros((B, T, DIM), dtype=np.float32)
    for c in range(N_CORES):
        ytc = res.results[c]["yt"].astype(np.float32)  # [NCH, DIM, 512]
        y[c // 2] += ytc.transpose(0, 2, 1).reshape(T, DIM)
    return y


# revision 34
# speedup vs baseline: 1.0575x; 1.0077x over previous
"""Causal self-attention Trainium2 Bass kernel.

Problem: B=4, T=2048, DIM=1024, H=16 heads, head_dim=64 (fp32).
  qkv = x @ w_qkv.T ; per-head causal softmax(q k^T / 8) v ; out @ w_out.T

Sharding (8 cores): core c -> (batch b = c//2, head-group g = c%2 of 8 heads).
Each core computes a partial output y_partial = attn_out_g @ w_out[:, g]^T
for its batch; host sums the two head-group partials per batch.

Device layout (per core):
  xt      [1024, 2048] bf16 : x[b]^T (dim-major)          -- host-transposed
  wqkvt   [1024, 1536] bf16 : [Wq|Wk|Wv]^T slice          -- host-transposed
  woutt   [ 512, 1024] bf16 : w_out[:, g]^T               -- host-transposed
  tri     [ 128,  128] bf16 : keep-triangle (1.0 iff q >= k within a block)
  yt      [1024, 2048] bf16 : partial output, transposed

Pipeline per token-chunk c (512 tokens), fully interleaved so PE keeps busy
while ScalarE runs the exp stream:
  1. QKV projection -> QT/KT (head-dim major, bf16) and V (token major, bf16,
     with a ones column per head that makes P@V also emit the softmax
     denominator row).
  2. Attention for q-chunk c: transposed scores for 2 heads per PSUM quad
     (row-packed via base_partition 0/64 so the K=64 matmuls run
     concurrently); one exp on ScalarE (scale=1/8 folded in, no
     max-subtraction; |scores| small so fp32 exp is safe); the 4 diagonal
     ktiles use partial-width scores/exp/PV (only q >= 128j columns) plus a
     128x128 triangle mask multiply, so ~53% instead of 62.5% of the full
     T^2 work is done; P@V accumulates per-head output plus denominator row;
     divide via fast-reciprocal off PSUM + GpSimd partition-broadcast +
     vector multiply (no DMA round-trip).
  3. Output projection of the finished 512-token chunk (bf16 out).
"""

import contextlib
import functools
import itertools

import numpy as np
import ml_dtypes

import concourse.bass as bass
import concourse.mybir as mybir
import concourse.tile as tile
from concourse import bacc
from concourse.bass_utils import run_bass_kernel_spmd

B, T, DIM = 4, 2048, 1024
NUM_HEADS, HEAD_DIM = 16, 64
INNER = NUM_HEADS * HEAD_DIM
SCALE = HEAD_DIM ** -0.5

N_CORES = 8
HEADS_PER_CORE = 8
HG = HEADS_PER_CORE * HEAD_DIM  # 512 = inner slice per core
NCH = T // 512                  # 4 token chunks
KT_PER_CH = 4                   # 128-ktok tiles per 512 chunk

F32 = mybir.dt.float32
BF16 = mybir.dt.bfloat16


def build_bass():
    nc = bacc.Bacc()
    # x/y are chunk-major [NCH, DIM, 512] so every [128, 512] tile transfer
    # is one contiguous 128KB block (vs 128 strided 1KB rows)
    xt = nc.declare_dram_parameter("xt", [NCH, DIM, 512], BF16, isOutput=False)
    # q/k/v-major so each [128, 512] slice is one contiguous 128KB block
    wqkvt = nc.declare_dram_parameter("wqkvt", [3, DIM, HG], BF16, isOutput=False)
    woutt = nc.declare_dram_parameter("woutt", [HG, DIM], BF16, isOutput=False)
    tri = nc.declare_dram_parameter("tri", [128, 128], BF16, isOutput=False)
    yt = nc.declare_dram_parameter("yt", [NCH, DIM, 512], BF16, isOutput=True)

    with tile.TileContext(nc) as tc:
        _emit(nc, tc, xt, wqkvt, woutt, tri, yt)
    nc.finalize()
    return nc


def _emit(nc, tc, xt, wqkvt, woutt, tri, yt):
    ctx = contextlib.ExitStack()
    with ctx:
        singles = ctx.enter_context(tc.tile_pool(name="singles", bufs=1))
        xpool = ctx.enter_context(tc.tile_pool(name="xpool", bufs=16))
        epool = ctx.enter_context(tc.tile_pool(name="epool", bufs=4))
        apool = ctx.enter_context(tc.tile_pool(name="apool", bufs=2))
        spool = ctx.enter_context(tc.tile_pool(name="spool", bufs=1))
        # PSUM budget (8 banks of 2KB/partition):
        #   pair [128,1024] bufs=2 -> 4 banks (scores, double-buffered)
        #   ot   [65,512]  2 slots -> 2 banks (otA/otB of the live pair; the
        #     next pair's PV start waits on this pair's divide, which the
        #     filler matmuls between pairs hide)
        #   qkv  [128,512] bufs=2  -> 2 banks (stage 1/3 groups double-buffered
        #     so group N+1's matmuls overlap group N's PSUM->SBUF copy)
        psq = ctx.enter_context(tc.tile_pool(name="psq", bufs=2, space="PSUM"))
        psot = ctx.enter_context(tc.tile_pool(name="psot", bufs=2, space="PSUM"))
        psmm = ctx.enter_context(tc.tile_pool(name="psmm", bufs=2, space="PSUM"))

        # ---- persistent SBUF tensors (wq first: they gate the first matmul).
        # wq[which][k] holds the q/k/v part of w slice k; the q parts + x
        # chunk 0 stream first (alternating over both HWDGE queues) so the
        # first projection group starts ~1us after the preamble.
        wq = [[singles.tile([128, HG], BF16, name=f"wq{w}_{k}") for k in range(8)]
              for w in range(3)]

        # QT/KT: 4 tiles [128, 2048] (2 heads per tile, head-dim major)
        qt = [singles.tile([128, T], BF16, name=f"qt{m}") for m in range(4)]
        kt = [singles.tile([128, T], BF16, name=f"kt{m}") for m in range(4)]
        # V: 16 token-tiles [128, 8*65] bf16 (per head: 64 v-cols + ones col)
        vt = [singles.tile([128, HEADS_PER_CORE * 65], BF16, name=f"vt{t}")
              for t in range(16)]

        def stage1_x(c, eng=None):
            xts = []
            for k in range(8):
                xtile = xpool.tile([128, 512], BF16, tag="xt", name=f"x{c}_{k}")
                (eng or nc.sync).dma_start(
                    out=xtile, in_=xt[c, k * 128:(k + 1) * 128, :])
                xts.append(xtile)
            return xts

        def stage1_qk(c, xts, which, m):
            cs = slice(c * 512, (c + 1) * 512)
            dst = qt if which == 0 else kt
            ps = psmm.tile([128, 512], F32, tag="qkv", name=f"pq{c}{which}{m}")
            for k in range(8):
                nc.tensor.matmul(
                    ps,
                    lhsT=wq[which][k][:, m * 128:(m + 1) * 128],
                    rhs=xts[k],
                    start=(k == 0), stop=(k == 7),
                )
            nc.vector.tensor_copy(dst[m][:, cs], ps)

        def stage1_v(c, xts, m):
            t = c * 4 + m
            ps = psmm.tile([128, 512], F32, tag="qkv", name=f"pv{t}")
            for k in range(8):
                nc.tensor.matmul(
                    ps,
                    lhsT=xts[k][:, m * 128:(m + 1) * 128],
                    rhs=wq[2][k],
                    start=(k == 0), stop=(k == 7),
                )
            v3 = vt[t].rearrange("p (h d) -> p h d", h=HEADS_PER_CORE)
            nc.vector.tensor_copy(
                v3[:, :, 0:64],
                ps.rearrange("p (h d) -> p h d", h=HEADS_PER_CORE))

        # startup: q-part weights + x chunk 0 first (alternating over both
        # HWDGE queues), then k parts, then v parts — matching the emission
        # order of chunk 0's projection groups (all q, all k, all v)
        xts0 = []
        for k in range(8):
            weng = nc.sync if k % 2 == 0 else nc.scalar
            xeng = nc.scalar if k % 2 == 0 else nc.sync
            weng.dma_start(out=wq[0][k], in_=wqkvt[0, k * 128:(k + 1) * 128, :])
            xtile = xpool.tile([128, 512], BF16, tag="xt", name=f"x0_{k}")
            xeng.dma_start(out=xtile, in_=xt[0, k * 128:(k + 1) * 128, :])
            xts0.append(xtile)
        for w in (1, 2):
            for k in range(8):
                eng = nc.sync if k % 2 == 0 else nc.scalar
                eng.dma_start(out=wq[w][k], in_=wqkvt[w, k * 128:(k + 1) * 128, :])
        for which in (0, 1):
            for m in range(4):
                stage1_qk(0, xts0, which, m)
        for m in range(4):
            stage1_v(0, xts0, m)

        trib = singles.tile([128, 128], BF16, name="trib")
        nc.scalar.dma_start(out=trib, in_=tri[:, :])
        wo = []
        for k in range(4):
            w = singles.tile([128, DIM], BF16, name=f"wo{k}")
            nc.scalar.dma_start(out=w, in_=woutt[k * 128:(k + 1) * 128, :])
            wo.append(w)
        # ones columns of V (denominator trick), written on-chip
        for t in range(16):
            v3 = vt[t].rearrange("p (h d) -> p h d", h=HEADS_PER_CORE)
            nc.vector.memset(v3[:, :, 64:65], 1.0)

        def attention(c, units, aot, cadence):
            """Emit attention for chunk c. `units` is a list of (min_pair, fn)
            small PE work units (single stage1/stage3 PSUM groups); one is
            injected every `cadence` ktiles so the PE always has dense matmul
            work queued while ScalarE's exp stream (983ns/tile) lags the
            score/PV production rate (~640ns/tile). PV is emitted one ktile
            behind scores so the PE never in-order-stalls on exp(tk)."""
            n_kt = KT_PER_CH * (c + 1)
            units = list(units)

            def pop_unit(hp):
                for idx, (mp, fn) in enumerate(units):
                    if mp <= hp:
                        units.pop(idx)
                        fn()
                        return True
                return False

            for hp in range(4):            # head pair (2hp, 2hp+1)
                hA, hB = 2 * hp, 2 * hp + 1
                otA = psot.tile([65, 512], F32, tag="ot", name=f"otA{c}_{hp}")
                otB = psot.tile([65, 512], F32, tag="ot", name=f"otB{c}_{hp}")
                pend = None       # (tk, e, q0) -> PV deferred one ktile
                since = 0

                def emit_pv(pv):
                    tk, e, q0 = pv
                    for i, ot in ((0, otA), (1, otB)):
                        nc.tensor.matmul(
                            ot[:, q0:512],
                            lhsT=vt[tk][:, (2 * hp + i) * 65:(2 * hp + i) * 65 + 65],
                            rhs=e[:, i * 512 + q0:(i + 1) * 512],
                            start=(tk == 0), stop=(tk == n_kt - 1),
                        )

                for tk in range(n_kt):  # pair: ktile tk x 2 heads
                    q = psq.tile([128, 1024], F32, tag="pair", name=f"s{c}_{hp}_{tk}")
                    # last 4 ktiles hit the causal diagonal: only columns
                    # q >= 128j are live -> partial-width scores/exp/PV
                    diag = tk >= n_kt - 4
                    j = tk - (n_kt - 4) if diag else 0
                    q0 = 128 * j            # first live q col within the chunk
                    for i in range(2):
                        ho = i * 64
                        nc.tensor.matmul(
                            q[:, i * 512 + q0:(i + 1) * 512],
                            lhsT=kt[hp][ho:ho + 64, tk * 128:(tk + 1) * 128],
                            rhs=qt[hp][ho:ho + 64, c * 512 + q0:(c + 1) * 512],
                            start=True, stop=True,
                            tile_position=(ho, 0),
                        )
                    e = epool.tile([128, 1024], BF16, tag="e", name=f"e{c}_{hp}_{tk}")
                    e3 = e.rearrange("p (h q) -> p h q", h=2)
                    q3 = q.rearrange("p (h q) -> p h q", h=2)
                    nc.scalar.activation(e3[:, :, q0:512], q3[:, :, q0:512],
                                         mybir.ActivationFunctionType.Exp,
                                         scale=float(SCALE))
                    if diag:
                        for i in range(2):
                            blk = slice(i * 512 + q0, i * 512 + q0 + 128)
                            nc.vector.tensor_mul(e[:, blk], e[:, blk], trib)
                    if pend is not None:
                        emit_pv(pend)
                    pend = (tk, e, q0)
                    since += 1
                    if since >= cadence and tk < n_kt - 1:
                        if pop_unit(hp):
                            since = 0
                emit_pv(pend)
                # divide: both heads' denominators gathered into one [1,1024]
                # tile -> single reciprocal + single GpSimd broadcast; the aot
                # multiplies land AFTER a filler unit so the DVE never
                # head-of-line blocks on the broadcast latency
                tail = c == NCH - 1 and hp == 3
                if tail:
                    pop_unit(hp)   # keep the unit's DVE copy off the critical chain
                den = spool.tile([1, 1024], F32, tag="den", name=f"dn{c}_{hp}")
                for i, ot in ((0, otA), (1, otB)):
                    if tail:
                        # ScalarE has drained its exp queue by now; shorten
                        # the tail-critical divide chain by copying there
                        nc.scalar.copy(den[:, i * 512:(i + 1) * 512], ot[64:65, :])
                    else:
                        nc.vector.tensor_copy(den[:, i * 512:(i + 1) * 512],
                                              ot[64:65, :])
                recf = spool.tile([1, 1024], F32, tag="recf", name=f"rf{c}_{hp}")
                nc.vector.reciprocal_approx_fast(recf, den)
                rb = spool.tile([64, 1024], F32, tag="rb", bufs=2,
                                name=f"rb{c}_{hp}")
                nc.gpsimd.partition_broadcast(rb, recf, channels=64)
                if not tail:
                    pop_unit(hp)
                for i, ot in ((0, otA), (1, otB)):
                    nc.vector.tensor_mul(
                        aot[hp][i * 64:i * 64 + 64, :],
                        ot[0:64, :], rb[:, i * 512:(i + 1) * 512])
            while units:
                units.pop(0)[1]()
            return aot

        def stage3_od(c, aot, od):
            ps = psmm.tile([128, 512], F32, tag="qkv", name=f"py{c}_{od}")
            for k in range(4):
                nc.tensor.matmul(
                    ps,
                    lhsT=wo[k][:, od * 128:(od + 1) * 128],
                    rhs=aot[k],
                    start=(k == 0), stop=(k == 3),
                )
            ys = spool.tile([128, 512], BF16, tag="ys", bufs=2, name=f"ys{c}_{od}")
            nc.vector.tensor_copy(ys, ps)
            nc.sync.dma_start(out=yt[c, od * 128:(od + 1) * 128, :], in_=ys)

        # Final-chunk output projection, split by contraction halves: the
        # k={0,1} half only needs head-pairs 0/1 of attention(NCH-1), so it
        # runs as filler behind pairs 2/3; only the k={2,3} half plus a DVE
        # combine remains after the last pair's divide.
        yp = [spool.tile([128, 512], F32, tag=f"yp{od}", name=f"yp{od}")
              for od in range(8)]

        def stage3_h1(c, aot, od):
            ps = psmm.tile([128, 512], F32, tag="qkv", name=f"pyA{c}_{od}")
            for k in (0, 1):
                nc.tensor.matmul(
                    ps,
                    lhsT=wo[k][:, od * 128:(od + 1) * 128],
                    rhs=aot[k],
                    start=(k == 0), stop=(k == 1),
                )
            nc.vector.tensor_copy(yp[od], ps)

        def stage3_h2(c, aot, i):
            cs = slice(c * 512, (c + 1) * 512)
            for od in (2 * i, 2 * i + 1):
                ps = psmm.tile([128, 512], F32, tag="qkv", name=f"pyB{c}_{od}")
                for k in (2, 3):
                    nc.tensor.matmul(
                        ps,
                        lhsT=wo[k][:, od * 128:(od + 1) * 128],
                        rhs=aot[k],
                        start=(k == 2), stop=(k == 3),
                    )
                ys = spool.tile([128, 512], BF16, tag="ys", bufs=2, name=f"ys{c}_{od}")
                nc.vector.scalar_tensor_tensor(
                    ys, ps, 1.0, yp[od],
                    op0=mybir.AluOpType.mult, op1=mybir.AluOpType.add)
                eng = nc.scalar if od % 2 == 1 else nc.sync
                eng.dma_start(out=yt[c, od * 128:(od + 1) * 128, :], in_=ys)

        # stage1(c+1) / stage3(c-1) PSUM groups are injected between ktiles of
        # attention(c) (cadence tuned per chunk so the filler rate matches the
        # exp-vs-PE deficit). h1 units carry min_pair=2: they read aot[0]/[1]
        # of the CURRENT chunk, which exist only once pairs 0/1 are divided.
        aot_prev = None
        cadence = [2, 2, 3, 4]
        for c in range(NCH):
            aot = [apool.tile([128, 512], BF16, tag=f"aot{k}", name=f"aot{c}_{k}")
                   for k in range(4)]
            u1, u3 = [], []
            if c + 1 < NCH:
                xts = stage1_x(c + 1)
                for m in range(4):
                    u1.append((0, functools.partial(stage1_qk, c + 1, xts, 0, m)))
                    u1.append((0, functools.partial(stage1_qk, c + 1, xts, 1, m)))
                    u1.append((0, functools.partial(stage1_v, c + 1, xts, m)))
            if aot_prev is not None:
                u3 = [(0, functools.partial(stage3_od, c - 1, aot_prev, od))
                      for od in range(8)]
            if c == NCH - 1:
                u3 += [(2, functools.partial(stage3_h1, c, aot, od))
                       for od in range(8)]
            units = [u for pair in itertools.zip_longest(u1, u3)
                     for u in pair if u is not None]
            attention(c, units, aot, cadence[c])
            aot_prev = aot
        for i in range(4):
            stage3_h2(NCH - 1, aot_prev, i)


_NC_CACHE = None


def _get_nc():
    global _NC_CACHE
    if _NC_CACHE is None:
        _NC_CACHE = build_bass()
    return _NC_CACHE


def make_tri():
    """Keep-triangle for the 128-wide diagonal blocks: tri[k, q] = 1 iff q >= k."""
    k = np.arange(128)[:, None]
    q = np.arange(128)[None, :]
    return (q >= k).astype(ml_dtypes.bfloat16)


def make_in_maps(x, w_qkv, w_out):
    x = np.asarray(x, dtype=np.float32)
    w_qkv = np.asarray(w_qkv, dtype=np.float32)
    w_out = np.asarray(w_out, dtype=np.float32)
    tri = make_tri()
    in_maps = []
    for c in range(N_CORES):
        b, g = c // 2, c % 2
        gs = slice(g * HG, (g + 1) * HG)
        wsel = np.concatenate(
            [w_qkv[0 * INNER:][gs], w_qkv[1 * INNER:][gs], w_qkv[2 * INNER:][gs]],
            axis=0)                               # [1536, 1024]
        xtc = x[b].T.reshape(DIM, NCH, 512).transpose(1, 0, 2)  # [NCH, DIM, 512]
        wq3 = np.stack([wsel[w * HG:(w + 1) * HG].T for w in range(3)])
        in_maps.append({
            "xt": np.ascontiguousarray(xtc).astype(ml_dtypes.bfloat16),
            "wqkvt": np.ascontiguousarray(wq3).astype(ml_dtypes.bfloat16),
            "woutt": np.ascontiguousarray(w_out[:, gs].T).astype(ml_dtypes.bfloat16),
            "tri": tri,
        })
    return in_maps


def kernel(x, mask, w_qkv, w_out, **_):
    nc = _get_nc()
    in_maps = make_in_maps(x, w_qkv, w_out)
    res = run_bass_kernel_spmd(nc, in_maps, core_ids=list(range(N_CORES)))
    y = np.zeros((B, T, DIM), dtype=np.float32)
    for c in range(N_CORES):
        ytc = res.results[c]["yt"].astype(np.float32)  # [NCH, DIM, 512]
        y[c // 2] += ytc.transpose(0, 2, 1).reshape(T, DIM)
    return y
